# revision 7
# baseline (speedup 1.0000x reference)
"""Trainium2 Bass kernel for nn_FGSAttn (frequency-gated spatial attention).

Self-contained: hardcodes shapes B=32, C=256, H=W=56, group=4, d=1 and the
pure-data-parallel sharding (batch / 8 cores). The FFT is computed as DFT
matmuls on the tensor engine; radial-bin pooling and the scatter back to
pixels are per-column matmuls against precomputed one-hot tables; the
channel max runs through PE transposes (with the channel mean fused into
the same matmul via an [I | ones/64] moving operand) + DVE segment reduces.
All matmuls use the single-pass float32r PE path.
"""
import numpy as np

import concourse.bass as bass
import concourse.bacc as bacc
import concourse.mybir as mybir
import concourse.tile as tile
from concourse import bass_utils

F32 = mybir.dt.float32
F32R = mybir.dt.float32r

H = 56
W = 56
HW = H * W
G = 4
KCH = 64          # channels per group
C = 256
NB = 29           # radial bins
BSH = 4           # batches per core
NCORE = 8
CH = 112          # px chunk for stats (2 rows of W)
NCH = HW // CH    # 28
NMAP = BSH * G    # 16 maps per core
CW = 130          # fused transpose+mean output width (128 T cols + 2 mean)


def _build_consts():
    """Host-precomputed constant tables shared by all cores (input-independent)."""
    jk = np.outer(np.arange(H), np.arange(H)).astype(np.float64)
    ang = -2.0 * np.pi * jk / H
    Fr = np.cos(ang).astype(np.float32)
    Fi = np.sin(ang).astype(np.float32)

    ch = H // 2
    hh = (np.arange(H) - ch).astype(np.float32)
    r = np.sqrt(hh[:, None] ** 2 + hh[None, :] ** 2)
    rf = np.minimum(np.floor(r), float(ch))
    lab = np.ceil(rf).astype(np.int32)
    lab0 = np.fft.ifftshift(lab)
    onehot0 = np.zeros((H, W, NB), np.float32)
    onehot0[np.arange(H)[:, None], np.arange(W)[None, :], lab0] = 1.0
    counts = onehot0.sum(axis=(0, 1))
    ohn = onehot0 / counts[None, None, :]

    rhsaug = np.zeros((128, CW), np.float32)
    rhsaug[:, 0:128] = np.eye(128, dtype=np.float32)
    rhsaug[0:64, 128] = 1.0 / KCH
    rhsaug[64:128, 129] = 1.0 / KCH

    poolw2 = np.zeros((112, 28 * NB), np.float32)
    for par in range(2):
        for vp in range(28):
            poolw2[56 * par:56 * par + 56, vp * NB:(vp + 1) * NB] = \
                ohn[:, 2 * vp + par, :]

    cst = {
        'FRFI': np.concatenate([Fr, Fi], axis=1),           # (56, 112)
        'Fer': np.ascontiguousarray(Fr[0::2, :]),
        'For': np.ascontiguousarray(Fr[1::2, :]),
        'Fei': np.ascontiguousarray(Fi[0::2, :]),
        'Foi': np.ascontiguousarray(Fi[1::2, :]),
        'nFei': np.ascontiguousarray(-Fi[0::2, :]),
        'nFoi': np.ascontiguousarray(-Fi[1::2, :]),
        'GAr': (Fr / H).astype(np.float32),
        'GAi': (-Fi / H).astype(np.float32),
        'nGAi': (Fi / H).astype(np.float32),
        'GBCAT': np.concatenate([Fr / H, Fi / H], axis=0),  # (112, 56)
        'POOLW2': poolw2,
        'GAINW': np.ascontiguousarray(onehot0.transpose(2, 1, 0).reshape(NB, W * H)),
        'RHSAUG': rhsaug,
        'ID56': np.eye(56, dtype=np.float32),
        'ONES': np.ones((1, HW), np.float32),
    }
    return cst


def _build_param_consts(gamma, fc_w, fc_b):
    """Constant tables derived from the (tiny) network parameters."""
    FCWT = np.ascontiguousarray(fc_w.transpose(2, 0, 1).reshape(NB, G * NB)).astype(np.float32)
    FCB = np.zeros((NB, NMAP), np.float32)
    for m in range(NMAP):
        FCB[:, m] = fc_b[m % G]
    GMASK5 = np.zeros((5, C), np.float32)
    for g in range(G):
        GMASK5[g, g * KCH:(g + 1) * KCH] = gamma[g * KCH:(g + 1) * KCH]
    GMASK5[4, :] = 1.0
    return {'FCWT': FCWT, 'FCB': FCB, 'GMASK5': GMASK5}


CONST_SHAPES = {
    'FRFI': (56, 112),
    'Fer': (28, 56), 'For': (28, 56), 'Fei': (28, 56), 'Foi': (28, 56),
    'nFei': (28, 56), 'nFoi': (28, 56),
    'GAr': (56, 56), 'GAi': (56, 56), 'nGAi': (56, 56),
    'GBCAT': (112, 56),
    'POOLW2': (112, 28 * 29), 'GAINW': (29, 56 * 56),
    'RHSAUG': (128, CW), 'ID56': (56, 56), 'ONES': (1, HW),
    'FCWT': (29, 4 * 29), 'FCB': (29, 16), 'GMASK5': (5, 256),
}

# constants consumed by float32r matmuls (DMA-loaded with that dtype)
R_CONSTS = {'FRFI', 'Fer', 'For', 'Fei', 'Foi', 'nFei', 'nFoi',
            'GAr', 'GAi', 'nGAi', 'GBCAT', 'POOLW2', 'GAINW', 'RHSAUG',
            'FCWT', 'GMASK5', 'ONES', 'ID56'}


def build_nc():
    nc = bacc.Bacc("TRN2", target_bir_lowering=False, debug=False,
                   num_devices=NCORE)
    feat_d = nc.dram_tensor("feature", (BSH, C, HW), F32, kind="ExternalInput").ap()
    out_d = nc.dram_tensor("out", (BSH, C, HW), F32, kind="ExternalOutput").ap()
    cd = {k: nc.dram_tensor(k, v, F32, kind="ExternalInput").ap()
          for k, v in CONST_SHAPES.items()}

    AL = mybir.AluOpType
    AX = mybir.AxisListType
    AF = mybir.ActivationFunctionType

    with tile.TileContext(nc) as tc:
        with (
            tc.tile_pool(name="cst", bufs=1) as cpool,
            tc.tile_pool(name="fbuf", bufs=1) as fpool,
            tc.tile_pool(name="work", bufs=1) as wpool,
            tc.tile_pool(name="attnp", bufs=2) as attnpool,
            tc.tile_pool(name="xsb", bufs=2) as xsbpool,
            tc.tile_pool(name="ps", bufs=8, space="PSUM") as psp,
        ):
            def pst(shape, name, dt=F32):
                return psp.tile(shape, dt, tag="psb", name=name,
                                padded_shape=[128, 512])

            # ---- constants into SBUF
            cs = {}
            for k, shp in CONST_SHAPES.items():
                if k == 'ONES':
                    continue
                dt_k = F32R if k in R_CONSTS else F32
                cs[k] = cpool.tile(list(shp), dt_k, tag=f"c_{k}", name=f"c_{k}")
                nc.sync.dma_start(cs[k][:, :],
                                  cd[k].bitcast(dt_k) if dt_k is F32R else cd[k])

            # ---- load feature tiles (128, 3136) as f32r and compute stats
            ftiles = {}
            for b in range(BSH):
                for ct in range(2):
                    f = fpool.tile([128, HW], F32R, tag=f"f{b}{ct}", name=f"f{b}{ct}")
                    ftiles[(b, ct)] = f
                    nc.sync.dma_start(f[:, :],
                                      feat_d[b, ct * 128:(ct + 1) * 128, :].bitcast(F32R))

            comps = {}
            comps_lo = {}
            for b in range(BSH):
                comp = wpool.tile([CH, NCH * G], F32R, tag=f"comp{b}", name=f"comp{b}")
                comp3 = comp.rearrange("p (c g) -> p c g", g=G)
                for ct in range(2):
                    f = ftiles[(b, ct)]
                    maxsb = wpool.tile([CH, NCH * 2], F32, tag="maxsb",
                                       name="maxsb", bufs=2)
                    max3 = maxsb.rearrange("p (c g) -> p c g", g=2)
                    meansb = wpool.tile([CH, NCH * 2], F32, tag="meansb",
                                        name="meansb", bufs=2)
                    mean3 = meansb.rearrange("p (c g) -> p c g", g=2)
                    # fused transpose+mean: out = f_chunk.T @ [I128 | blk/64]
                    for cgrp in range(10):   # 9 tiles x 3 chunks + 1 x 1
                        nk = 3 if cgrp < 9 else 1
                        T = pst([CH, nk * CW], "Tps")
                        for k in range(nk):
                            c = cgrp * 3 + k
                            nc.tensor.matmul(
                                T[:, k * CW:(k + 1) * CW],
                                lhsT=f[:, c * CH:(c + 1) * CH],
                                rhs=cs['RHSAUG'][:, :])
                        Tv = T.rearrange("p (k w) -> p k w", w=CW)
                        nc.vector.tensor_reduce(
                            max3[:, cgrp * 3:cgrp * 3 + nk, :],
                            Tv[:, :, 0:128].rearrange("p k (g s) -> p k g s", s=64),
                            axis=AX.X, op=AL.max)
                        nc.scalar.copy(mean3[:, cgrp * 3:cgrp * 3 + nk, :],
                                       Tv[:, :, 128:130])
                    # comp[:, :, 2ct:2ct+2] = max + mean
                    nc.vector.tensor_tensor(
                        comp3[:, :, 2 * ct:2 * ct + 2],
                        max3[:, :, :], mean3[:, :, :], op=AL.add)
                comps[b] = comp
                # partition-shifted copy of rows 56:112 (odd image rows)
                clo = wpool.tile([56, NCH * G], F32R, tag=f"complo{b}",
                                 name=f"complo{b}")
                nc.scalar.dma_start(clo[:, :], comp[56:CH, :])
                comps_lo[b] = clo

            # ---- forward FFT stage 1 (contract w): per (map, h-parity):
            # out (28, [Fr-out | Fi-out]) with rhs = [Fr | Fi]
            t1sb = {}
            for i, kind in enumerate(('er', 'ei', 'or', 'oi')):
                t1sb[kind] = wpool.tile([28, NMAP * 56], F32R, tag=f"big{i}",
                                        name=f"t1{kind}",
                                        padded_shape=[128, NMAP * 56])
            for b in range(BSH):
                comp3 = comps[b].rearrange("p (c g) -> p c g", g=G)
                clo3 = comps_lo[b].rearrange("p (c g) -> p c g", g=G)
                for par, csrc in ((0, comp3), (1, clo3)):
                    t1p = pst([28, G * 112], f"pt1{par}")
                    for g in range(G):
                        nc.tensor.matmul(
                            t1p[:, g * 112:(g + 1) * 112],
                            lhsT=csrc[0:56, :, g], rhs=cs['FRFI'][:, :])
                    # copy psum (28,[g][ri][v]) -> sbuf (28,[v][m=4b+g])
                    src = t1p.rearrange("p (g ri v) -> p ri v g", g=G, ri=2)
                    for ri, kind in ((0, 'er' if par == 0 else 'or'),
                                     (1, 'ei' if par == 0 else 'oi')):
                        dst = t1sb[kind].rearrange("p (v m) -> p v m", m=NMAP)
                        nc.scalar.copy(dst[:, :, 4 * b:4 * b + 4],
                                       src[:, ri, :, :])

            # ---- stage 2 (contract h): spec halves (56, 448)
            spec = {}
            for ri in ('r', 'i'):
                for h in range(2):
                    spec[(ri, h)] = pst([56, 448], f"spec{ri}{h}")
            for h in range(2):
                sl = slice(h * 448, (h + 1) * 448)
                for ri, terms in (
                        ('r', (('Fer', 'er'), ('For', 'or'),
                               ('nFei', 'ei'), ('nFoi', 'oi'))),
                        ('i', (('Fer', 'ei'), ('For', 'oi'),
                               ('Fei', 'er'), ('Foi', 'or')))):
                    for i, (wname, kind) in enumerate(terms):
                        nc.tensor.matmul(
                            spec[(ri, h)][:, :],
                            lhsT=cs[wname][:, :],
                            rhs=t1sb[kind][:, sl],
                            start=(i == 0), stop=(i == 3))

            # ---- amplitude, directly in the v-parity-stacked (112, 448) form
            sq = wpool.tile([56, 896], F32, tag="tmqa", name="sq")
            sq2 = wpool.tile([56, 896], F32, tag="tmqb", name="sq2")
            for h in range(2):
                sl = slice(h * 448, (h + 1) * 448)
                nc.scalar.square(sq[:, sl], spec[('r', h)][:, :])
                nc.scalar.square(sq2[:, sl], spec[('i', h)][:, :])
            nc.vector.tensor_tensor(sq[:, :], sq[:, :], sq2[:, :], op=AL.add)
            sqv = sq.rearrange("p (vp par m) -> p vp par m", par=2, m=NMAP)
            amp2 = wpool.tile([112, 448], F32R, tag="amp2", name="amp2")
            stg = wpool.tile([56, 448], F32R, tag="ampstg", name="ampstg")
            a2v = amp2.rearrange("p (vp m) -> p vp m", m=NMAP)
            sgv = stg.rearrange("p (vp m) -> p vp m", m=NMAP)
            nc.scalar.sqrt(a2v[0:56, :, :], sqv[:, :, 0, :])
            nc.scalar.sqrt(sgv[:, :, :], sqv[:, :, 1, :])
            nc.scalar.dma_start(amp2[56:112, :], stg[:, :])

            # ---- radial pooling: 28 accumulating matmuls -> pooled (29, 16)
            pooled_ps = pst([29, NMAP], "pooled")
            for vp in range(28):
                nc.tensor.matmul(
                    pooled_ps[:, :],
                    lhsT=cs['POOLW2'][:, vp * NB:(vp + 1) * NB],
                    rhs=amp2[:, vp * NMAP:(vp + 1) * NMAP],
                    start=(vp == 0), stop=(vp == 27))
            pooled_sb = wpool.tile([29, NMAP], F32R, tag="pooledsb", name="pooledsb")
            nc.scalar.copy(pooled_sb[:, :], pooled_ps[:, :])

            # ---- per-group FC + leaky relu -> att (29, 16)
            attp = wpool.tile([29, NMAP], F32, tag="attp", name="attp")
            attp3 = attp.rearrange("p (b g) -> p b g", g=G)
            pooled3 = pooled_sb.rearrange("p (b g) -> p b g", g=G)
            for g in range(G):
                fc_ps = pst([29, BSH], "fcps")
                nc.tensor.matmul(
                    fc_ps[:, :],
                    lhsT=cs['FCWT'][:, g * NB:(g + 1) * NB],
                    rhs=pooled3[:, :, g])
                nc.scalar.copy(attp3[:, :, g], fc_ps[:, :])
            att = wpool.tile([29, NMAP], F32R, tag="att", name="att")
            att2 = wpool.tile([29, NMAP], F32, tag="att2", name="att2")
            nc.vector.tensor_tensor(att[:, :], attp[:, :], cs['FCB'][:, :], op=AL.add)
            nc.scalar.mul(att2[:, :], att[:, :], 0.01)
            nc.vector.tensor_tensor(att[:, :], att[:, :], att2[:, :], op=AL.max)

            # ---- gain maps: per-v matmuls -> gain (56, [v][m]) halves
            gain_ps = {h: pst([56, 448], f"gain{h}") for h in range(2)}
            for v in range(W):
                h, vv = (0, v) if v < 28 else (1, v - 28)
                nc.tensor.matmul(
                    gain_ps[h][:, vv * NMAP:(vv + 1) * NMAP],
                    lhsT=cs['GAINW'][:, v * 56:(v + 1) * 56],
                    rhs=att[:, :])
            gain_sb = wpool.tile([56, 896], F32, tag="big2", name="gainsb")
            for h in range(2):
                nc.scalar.copy(gain_sb[:, h * 448:(h + 1) * 448], gain_ps[h][:, :])

            # ---- M = spec * gain (SBUF; reuses sq/sq2 slots)
            Msb = {'r': wpool.tile([56, 896], F32R, tag="tmqa", name="Mr"),
                   'i': wpool.tile([56, 896], F32R, tag="tmqb", name="Mi")}
            for ri in ('r', 'i'):
                for h in range(2):
                    sl = slice(h * 448, (h + 1) * 448)
                    nc.vector.tensor_tensor(
                        Msb[ri][:, sl], spec[(ri, h)][:, :], gain_sb[:, sl],
                        op=AL.mult)

            # ---- inverse stage A: Q = conj(F)/56 @ M
            Qps = {}
            for ri in ('r', 'i'):
                for h in range(2):
                    Qps[(ri, h)] = pst([56, 448], f"Q{ri}{h}")
            for h in range(2):
                for ri, terms in (
                        ('r', (('GAr', 'r'), ('nGAi', 'i'))),
                        ('i', (('GAr', 'i'), ('GAi', 'r')))):
                    for i, (wname, mk) in enumerate(terms):
                        nc.tensor.matmul(
                            Qps[(ri, h)][:, :],
                            lhsT=cs[wname][:, :],
                            rhs=Msb[mk][:, h * 448:(h + 1) * 448],
                            start=(i == 0), stop=(i == 1))
            # pack Q into (56, [m][ri][v]) so each map is one contiguous lhsT
            Qcat = wpool.tile([56, NMAP * 112], F32R, tag="qcat", name="Qcat")
            Qc4 = Qcat.rearrange("p (m ri v) -> p m ri v", ri=2, v=56)
            for ri_i, ri in enumerate(('r', 'i')):
                for h in range(2):
                    src = Qps[(ri, h)].rearrange("p (v m) -> p m v", m=NMAP)
                    nc.scalar.copy(Qc4[:, :, ri_i, h * 28:(h + 1) * 28], src)

            # ---- inverse stage B: per map one transpose + one matmul
            nf_ps = {h: pst([56, 448], f"nf{h}") for h in range(2)}
            for m in range(NMAP):
                qt_ps = pst([112, 56], "qtps", dt=F32R)
                nc.tensor.transpose(qt_ps[:, :],
                                    Qcat[:, m * 112:(m + 1) * 112],
                                    cs['ID56'][:, :])
                qt_sb = wpool.tile([112, 56], F32R, tag="qtsb", name="qtsb", bufs=3)
                nc.scalar.copy(qt_sb[:, :], qt_ps[:, :])
                h, mm_ = (0, m) if m < 8 else (1, m - 8)
                nc.tensor.matmul(nf_ps[h][:, mm_ * 56:(mm_ + 1) * 56],
                                 lhsT=qt_sb[:, :], rhs=cs['GBCAT'][:, :])

            # ---- per-map min/max -> scale/bias (4, 4)
            red = wpool.tile([56, 64], F32R, tag="red", name="red")
            nc.vector.memset(red[:, :].bitcast(F32), 0.0)
            for h in range(2):
                nf3 = nf_ps[h].rearrange("p (m w) -> p m w", w=56)
                nc.vector.tensor_reduce(red[:, h * 8:(h + 1) * 8], nf3,
                                        axis=AX.X, op=AL.min)
                nc.vector.tensor_reduce(red[:, 32 + h * 8:32 + (h + 1) * 8], nf3,
                                        axis=AX.X, op=AL.max)
            sc_ps = pst([64, 56], "scps", dt=F32R)
            nc.tensor.transpose(sc_ps[:, :], red[:, :], cs['ID56'][:, :])
            sc_sb = wpool.tile([64, 56], F32, tag="scsb", name="scsb")
            nc.scalar.copy(sc_sb[:, :], sc_ps[:, :])
            scarr = wpool.tile([64, 1], F32, tag="scarr", name="scarr")
            nc.vector.tensor_reduce(scarr[0:32, :], sc_sb[0:32, :],
                                    axis=AX.X, op=AL.min)
            nc.vector.tensor_reduce(scarr[32:64, :], sc_sb[32:64, :],
                                    axis=AX.X, op=AL.max)
            mn44 = wpool.tile([4, 4], F32, tag="mn44", name="mn44")
            mx44 = wpool.tile([4, 4], F32, tag="mx44", name="mx44")
            for b in range(BSH):
                nc.scalar.dma_start(mn44[:, b:b + 1], scarr[4 * b:4 * b + 4, :])
                nc.scalar.dma_start(mx44[:, b:b + 1],
                                    scarr[32 + 4 * b:36 + 4 * b, :])
            s44 = wpool.tile([4, 4], F32, tag="s44", name="s44")
            b44 = wpool.tile([4, 4], F32, tag="b44", name="b44")
            nc.vector.tensor_tensor(s44[:, :], mx44[:, :], mn44[:, :],
                                    op=AL.subtract)
            nc.vector.reciprocal(s44[:, :], s44[:, :])
            nc.vector.tensor_tensor(b44[:, :], mn44[:, :], s44[:, :], op=AL.mult)
            nc.scalar.mul(b44[:, :], b44[:, :], -1.0)

            # ---- nf psum -> sbuf (big3 slot)
            nf_sb = wpool.tile([56, 896], F32R, tag="big3", name="nfsb")
            for h in range(2):
                nc.scalar.copy(nf_sb[:, h * 448:(h + 1) * 448], nf_ps[h][:, :])

            # ---- per b: attn rows, normalize, X = GMASK5.T @ attn, apply, store
            nchunks = [(i * 512, min(512, HW - i * 512))
                       for i in range((HW + 511) // 512)]
            for b in range(BSH):
                ab = attnpool.tile([5, HW], F32R, tag="attnb", name=f"attn{b}")
                nc.sync.dma_start(ab[4:5, :], cd['ONES'].bitcast(F32R))
                for g in range(G):
                    m = 4 * b + g
                    nc.gpsimd.dma_start(ab[g:g + 1, :],
                                        nf_sb[:, m * 56:(m + 1) * 56])
                nc.scalar.activation(ab[0:4, :], ab[0:4, :], AF.Identity,
                                     bias=b44[:, b:b + 1], scale=s44[:, b:b + 1])
                for ct in range(2):
                    f = ftiles[(b, ct)]
                    for ci, (off, n) in enumerate(nchunks):
                        X = pst([128, 512], "Xps")
                        nc.tensor.matmul(
                            X[:, 0:n],
                            lhsT=cs['GMASK5'][:, ct * 128:(ct + 1) * 128],
                            rhs=ab[0:5, off:off + n])
                        if ci in (2, 5):
                            xs = xsbpool.tile([128, 512], F32, tag="xs", name="xs")
                            nc.scalar.copy(xs[:, 0:n], X[:, 0:n])
                            nc.gpsimd.tensor_tensor(
                                f[:, off:off + n], f[:, off:off + n],
                                xs[:, 0:n], op=AL.mult)
                        else:
                            nc.vector.tensor_tensor(
                                f[:, off:off + n], f[:, off:off + n], X[:, 0:n],
                                op=AL.mult)
                    nc.sync.dma_start(out_d[b, ct * 128:(ct + 1) * 128, :],
                                      f[:, :].bitcast(F32))
    nc.compile()
    return nc


_NC_CACHE = {}


def _get_nc():
    if 'nc' not in _NC_CACHE:
        _NC_CACHE['nc'] = build_nc()
    return _NC_CACHE['nc']


def kernel(feature, gamma, fc_w, fc_b, group=None, d=None, **kw):
    feature = np.ascontiguousarray(np.asarray(feature), dtype=np.float32)
    gamma = np.asarray(gamma, dtype=np.float32)
    fc_w = np.asarray(fc_w, dtype=np.float32)
    fc_b = np.asarray(fc_b, dtype=np.float32)
    B = feature.shape[0]

    cst = _build_consts()
    cst.update(_build_param_consts(gamma, fc_w, fc_b))
    cst = {k: np.ascontiguousarray(v, dtype=np.float32) for k, v in cst.items()}

    featr = feature.reshape(B, C, HW)
    in_maps = []
    for c in range(NCORE):
        m = dict(cst)
        m['feature'] = np.ascontiguousarray(featr[c * BSH:(c + 1) * BSH])
        in_maps.append(m)

    nc = _get_nc()
    res = bass_utils.run_bass_kernel_spmd(nc, in_maps, core_ids=list(range(NCORE)))
    out = np.concatenate([res.results[c]['out'] for c in range(NCORE)], axis=0)
    return out.reshape(B, C, H, W).astype(np.float32)
